# revision 7
# baseline (speedup 1.0000x reference)
"""CodebookEMA (VQ) Trainium2 kernel — 8-core data-parallel over tokens.

Strategy (per sharding hint): shard the BT=65536 token axis 8192/core,
replicate the [D=256, K=1024] codebook, AllReduce per-core EMA stats
(counts [K] + embedding sums [K,D]), then each core computes the updated
codebook and gathers its own z_q shard. Host concatenates shards.

Distances use an fp16 hi/lo 3-matmul split (f32-grade, ~1e-5 abs err vs the
1.6e-4 min argmin gap), since plain fp32 matmuls are 4x slower and f32r
rounds inputs to ~12 bits. One-hot stats matmuls run in fp16 (mask is exact
0/1; z fp16 error is scaled by 0.01 in the EMA update).
"""
import sys

sys.path.insert(0, "/opt/trn_rl_repo")

import numpy as np
import concourse.bass as bass
import concourse.bacc as bacc
import concourse.tile as tile
from concourse import mybir, bass_isa
from concourse.bass_utils import run_bass_kernel_spmd
from concourse.masks import make_identity

F32 = mybir.dt.float32
F16 = mybir.dt.float16
I32 = mybir.dt.int32
AX = mybir.AxisListType
OP = mybir.AluOpType
AF = mybir.ActivationFunctionType

NCORES = 8
B, T, D, K = 16, 4096, 256, 1024
BT = B * T
TPC = BT // NCORES          # 8192 tokens per core
NT = TPC // 128             # 64 token tiles
TPB = T // TPC if T >= TPC else 0
BATCHES_PER_CORE = TPC // T  # 2 full batches per core
DECAY = 0.99
ONE_MINUS_DECAY = 1.0 - 0.99
EPS = 1e-05
KEPS = float(np.float32(K * 1e-05))

_CACHE = {}


def _build():
    nc = bacc.Bacc("TRN2", target_bir_lowering=False, debug=False,
                   num_devices=NCORES)

    # ---------------- I/O ----------------
    zt_hi_in = nc.declare_dram_parameter("zt_hi", [D, TPC], F16, isOutput=False)
    zt_lo_in = nc.declare_dram_parameter("zt_lo", [D, TPC], F16, isOutput=False)
    z16_in = nc.declare_dram_parameter("z16", [TPC, D], F16, isOutput=False)
    e_hi_in = nc.declare_dram_parameter("e_hi", [D, K], F16, isOutput=False)
    e_lo_in = nc.declare_dram_parameter("e_lo", [D, K], F16, isOutput=False)
    e2_hi_in = nc.declare_dram_parameter("e2_hi", [1, K], F16, isOutput=False)
    e2_lo_in = nc.declare_dram_parameter("e2_lo", [1, K], F16, isOutput=False)
    emt_in = nc.declare_dram_parameter("emt", [K, D], F32, isOutput=False)
    cs_in = nc.declare_dram_parameter("cs", [1, K], F32, isOutput=False)

    zq_out = nc.declare_dram_parameter("zq", [TPC, D], F32, isOutput=True)
    code_out = nc.declare_dram_parameter("code", [NT, 128], I32, isOutput=True)
    closs_out = nc.declare_dram_parameter("closs", [1, BATCHES_PER_CORE], F32,
                                          isOutput=True)
    newE_out = nc.declare_dram_parameter("newE", [D, K], F32, isOutput=True)
    nem_out = nc.declare_dram_parameter("nem", [D, K], F32, isOutput=True)
    ncs_out = nc.declare_dram_parameter("ncs", [1, K], F32, isOutput=True)

    # internal DRAM
    table = nc.dram_tensor("table", [K, D], F32)  # updated codebook, [K,D]
    AR_N = K * D + K
    ar_in = nc.dram_tensor("ar_in", [1, AR_N], F32)
    ar_out = nc.dram_tensor("ar_out", [1, AR_N], F32, addr_space="Shared")

    with tile.TileContext(nc) as tc:
        with (
            tc.tile_pool(name="res", bufs=1) as res,       # resident tiles
            tc.tile_pool(name="work", bufs=2) as work,     # per-tile work
            tc.tile_pool(name="ps_sc", bufs=2, space="PSUM") as ps_sc,
            tc.tile_pool(name="ps_acc", bufs=1, space="PSUM") as ps_acc,
            tc.tile_pool(name="tail", bufs=3) as tailp,
        ):
            # ---------- constants ----------
            ones_col = res.tile([128, 1], F16, tag="ones_col")
            nc.gpsimd.memset(ones_col[:], 1.0)
            ones_row = res.tile([1, 128], F16, tag="ones_row")
            nc.gpsimd.memset(ones_row[:], 1.0)
            iota32 = res.tile([128, K], I32, tag="iota32")
            nc.gpsimd.iota(iota32[:], pattern=[[1, K]], base=0,
                           channel_multiplier=0)
            iota16 = res.tile([128, K], F16, tag="iota16")
            nc.vector.tensor_copy(out=iota16[:], in_=iota32[:])
            ident = res.tile([128, 128], F32, tag="ident")
            make_identity(nc, ident[:])

            # ---------- resident loads ----------
            zt_hi = res.tile([128, 2 * TPC], F16, tag="zt_hi")
            zt_lo = res.tile([128, 2 * TPC], F16, tag="zt_lo")
            for dc in range(2):
                rs = slice(dc * 128, (dc + 1) * 128)
                nc.sync.dma_start(zt_hi[:, dc * TPC:(dc + 1) * TPC], zt_hi_in[rs, :])
                nc.sync.dma_start(zt_lo[:, dc * TPC:(dc + 1) * TPC], zt_lo_in[rs, :])
            ehi = res.tile([128, 2 * K], F16, tag="ehi")
            elo = res.tile([128, 2 * K], F16, tag="elo")
            for dc in range(2):
                rs = slice(dc * 128, (dc + 1) * 128)
                nc.sync.dma_start(ehi[:, dc * K:(dc + 1) * K], e_hi_in[rs, :])
                nc.sync.dma_start(elo[:, dc * K:(dc + 1) * K], e_lo_in[rs, :])
            e2hi = res.tile([1, K], F16, tag="e2hi")
            e2lo = res.tile([1, K], F16, tag="e2lo")
            nc.sync.dma_start(e2hi[:], e2_hi_in[:])
            nc.sync.dma_start(e2lo[:], e2_lo_in[:])
            # z16 resident as [128 part, NT*D]: token p of tile t at [p, t*D:(t+1)*D]
            z16r = res.tile([128, NT * D], F16, tag="z16r")
            nc.sync.dma_start(
                z16r[:].rearrange("p (n d) -> p n d", d=D),
                z16_in[:].rearrange("(n p) d -> p n d", p=128),
            )

            idx_all = res.tile([128, NT], F32, tag="idx_all")

            # emb accumulators: 4 banks, 2 K-chunks each [128,(256|256)]
            emb_ps = [ps_acc.tile([128, 512], F32, space="PSUM", tag=f"emb{b}", name=f"emb{b}")
                      for b in range(4)]
            cnt_ps = [ps_acc.tile([1, 512], F32, space="PSUM", tag=f"cnt{b}", name=f"cnt{b}")
                      for b in range(2)]

            # ---------- main loop ----------
            for t in range(NT):
                scores = work.tile([128, K], F32, tag="scores")
                for kc in range(2):
                    sc_ps = ps_sc.tile([128, 512], F32, space="PSUM", tag="sc")
                    ks = slice(kc * 512, (kc + 1) * 512)
                    for dc in range(2):
                        zs = slice(dc * TPC + t * 128, dc * TPC + (t + 1) * 128)
                        es = slice(dc * K + kc * 512, dc * K + (kc + 1) * 512)
                        nc.tensor.matmul(out=sc_ps[:], lhsT=zt_hi[:, zs],
                                         rhs=ehi[:, es], start=(dc == 0), stop=False)
                        nc.tensor.matmul(out=sc_ps[:], lhsT=zt_hi[:, zs],
                                         rhs=elo[:, es], start=False, stop=False)
                        nc.tensor.matmul(out=sc_ps[:], lhsT=zt_lo[:, zs],
                                         rhs=ehi[:, es], start=False, stop=False)
                    nc.tensor.matmul(out=sc_ps[:], lhsT=ones_row[:],
                                     rhs=e2hi[:1, ks], start=False, stop=False)
                    nc.tensor.matmul(out=sc_ps[:], lhsT=ones_row[:],
                                     rhs=e2lo[:1, ks], start=False, stop=True)
                    nc.scalar.copy(out=scores[:, ks], in_=sc_ps[:])

                minv = work.tile([128, 1], F32, tag="minv")
                nc.vector.tensor_reduce(out=minv[:], in_=scores[:], axis=AX.X,
                                        op=OP.min)
                mask = work.tile([128, K], F16, tag="mask")
                nc.vector.tensor_scalar(out=mask[:], in0=scores[:],
                                        scalar1=minv[:], scalar2=None,
                                        op0=OP.is_equal)
                junk = work.tile([128, 1], F32, tag="junk")
                nc.vector.affine_mul_reduce(
                    out=junk[:].broadcast_to((128, K)), in0=mask[:],
                    in1=iota16[:], scale=1.0, bias=0.0,
                    accum_out=idx_all[:, t:t + 1],
                )
                # stats matmuls
                for c in range(8):
                    nc.tensor.matmul(
                        out=emb_ps[c // 2][:, (c % 2) * 256:(c % 2 + 1) * 256],
                        lhsT=mask[:, c * 128:(c + 1) * 128],
                        rhs=z16r[:, t * D:(t + 1) * D],
                        start=(t == 0), stop=(t == NT - 1),
                    )
                for kc in range(2):
                    nc.tensor.matmul(
                        out=cnt_ps[kc][:1, :],
                        lhsT=ones_col[:],
                        rhs=mask[:, kc * 512:(kc + 1) * 512],
                        start=(t == 0), stop=(t == NT - 1),
                    )

            # ---------- stats -> DRAM -> AllReduce ----------
            ar_view = ar_in[:1, :K * D].rearrange("a (k d) -> (a k) d", d=D)
            ar_out_view = ar_out[:1, :K * D].rearrange("a (k d) -> (a k) d", d=D)
            for b in range(4):
                emb_sb = work.tile([128, 512], F32, tag="emb_sb")
                nc.scalar.copy(out=emb_sb[:], in_=emb_ps[b][:])
                for h in range(2):
                    c = 2 * b + h
                    nc.sync.dma_start(
                        ar_view[c * 128:(c + 1) * 128, :],
                        emb_sb[:, h * 256:(h + 1) * 256],
                    )
            cnt_sb = res.tile([1, K], F32, tag="cnt_sb")
            for kc in range(2):
                nc.scalar.copy(out=cnt_sb[:1, kc * 512:(kc + 1) * 512],
                               in_=cnt_ps[kc][:1, :])
            nc.sync.dma_start(ar_in[:1, K * D:], cnt_sb[:])

            nc.gpsimd.collective_compute(
                "AllReduce", OP.add,
                replica_groups=[list(range(NCORES))],
                ins=[ar_in[:]], outs=[ar_out[:]],
            )

            # ---------- codebook update ----------
            counts = res.tile([1, K], F32, tag="counts")
            nc.sync.dma_start(counts[:], ar_out[:1, K * D:])
            counts_ch = res.tile([128, 8], F32, tag="counts_ch")
            nc.sync.dma_start(
                counts_ch[:, None, :],
                ar_out[:1, K * D:].rearrange("a (c p) -> p a c", p=128),
            )
            cs_old = res.tile([1, K], F32, tag="cs_old")
            nc.sync.dma_start(cs_old[:], cs_in[:])
            cs_ch = res.tile([128, 8], F32, tag="cs_ch")
            nc.sync.dma_start(cs_ch[:, None, :],
                              cs_in[:1, :].rearrange("a (c p) -> p a c", p=128))

            # new_cluster_size, flat layout (for output + n)
            ncs = res.tile([1, K], F32, tag="ncs")
            tmpk = res.tile([1, K], F32, tag="tmpk")
            nc.vector.tensor_scalar(out=ncs[:], in0=cs_old[:], scalar1=DECAY,
                                    scalar2=None, op0=OP.mult)
            nc.vector.tensor_scalar(out=tmpk[:], in0=counts[:],
                                    scalar1=ONE_MINUS_DECAY, scalar2=None,
                                    op0=OP.mult)
            nc.vector.tensor_tensor(out=ncs[:], in0=ncs[:], in1=tmpk[:], op=OP.add)
            nc.sync.dma_start(ncs_out[:], ncs[:])
            # chunked layout [128, 8]
            ncs_ch = res.tile([128, 8], F32, tag="ncs_ch")
            tmp8 = res.tile([128, 8], F32, tag="tmp8")
            nc.vector.tensor_scalar(out=ncs_ch[:], in0=cs_ch[:], scalar1=DECAY,
                                    scalar2=None, op0=OP.mult)
            nc.vector.tensor_scalar(out=tmp8[:], in0=counts_ch[:],
                                    scalar1=ONE_MINUS_DECAY, scalar2=None,
                                    op0=OP.mult)
            nc.vector.tensor_tensor(out=ncs_ch[:], in0=ncs_ch[:], in1=tmp8[:],
                                    op=OP.add)

            # n = sum(ncs); broadcast to all partitions
            n11 = res.tile([1, 1], F32, tag="n11")
            nc.vector.tensor_reduce(out=n11[:], in_=ncs[:], axis=AX.X, op=OP.add)
            nbc = res.tile([128, 1], F32, tag="nbc")
            nc.gpsimd.partition_broadcast(nbc[:], n11[:])
            denom = res.tile([128, 1], F32, tag="denom")
            nc.vector.tensor_scalar(out=denom[:], in0=nbc[:], scalar1=KEPS,
                                    scalar2=None, op0=OP.add)
            r1 = res.tile([128, 1], F32, tag="r1")
            nc.vector.reciprocal(r1[:], denom[:])
            # cs_norm = ((ncs + EPS) * r1) * n   per chunk [128,8]
            csn = res.tile([128, 8], F32, tag="csn")
            nc.vector.tensor_scalar(out=csn[:], in0=ncs_ch[:], scalar1=EPS,
                                    scalar2=r1[:], op0=OP.add, op1=OP.mult)
            nc.vector.tensor_scalar(out=csn[:], in0=csn[:], scalar1=nbc[:],
                                    scalar2=None, op0=OP.mult)
            rcs = res.tile([128, 8], F32, tag="rcs")
            nc.vector.reciprocal(rcs[:], csn[:])

            newET = res.tile([128, 2 * K], F32, tag="newET")   # [d, K] x2 chunks
            nemT = res.tile([128, 2 * K], F32, tag="nemT")
            for c in range(8):
                rs = slice(c * 128, (c + 1) * 128)
                em_old = tailp.tile([128, D], F32, tag="em_old")
                nc.sync.dma_start(em_old[:], emt_in[rs, :])
                es_sum = tailp.tile([128, D], F32, tag="es_sum")
                nc.sync.dma_start(es_sum[:], ar_out_view[rs, :])
                nem = tailp.tile([128, D], F32, tag="nem")
                tmpd = tailp.tile([128, D], F32, tag="tmpd")
                nc.vector.tensor_scalar(out=nem[:], in0=em_old[:], scalar1=DECAY,
                                        scalar2=None, op0=OP.mult)
                nc.vector.tensor_scalar(out=tmpd[:], in0=es_sum[:],
                                        scalar1=ONE_MINUS_DECAY, scalar2=None,
                                        op0=OP.mult)
                nc.vector.tensor_tensor(out=nem[:], in0=nem[:], in1=tmpd[:],
                                        op=OP.add)
                newE = tailp.tile([128, D], F32, tag="newE")
                nc.vector.tensor_scalar(out=newE[:], in0=nem[:],
                                        scalar1=rcs[:, c:c + 1], scalar2=None,
                                        op0=OP.mult)
                nc.sync.dma_start(table[rs, :], newE[:])
                # transpose both [K,D] results into [D,K] output layout
                for src, dstt in ((newE, newET), (nem, nemT)):
                    for dc in range(2):
                        tp = ps_sc.tile([128, 512], F32, space="PSUM", tag="sc")
                        nc.tensor.transpose(out=tp[:, :128],
                                            in_=src[:, dc * 128:(dc + 1) * 128],
                                            identity=ident[:])
                        nc.scalar.copy(
                            out=dstt[:, dc * K + c * 128: dc * K + (c + 1) * 128],
                            in_=tp[:, :128])
            for dc in range(2):
                rs = slice(dc * 128, (dc + 1) * 128)
                nc.sync.dma_start(newE_out[rs, :], newET[:, dc * K:(dc + 1) * K])
                nc.sync.dma_start(nem_out[rs, :], nemT[:, dc * K:(dc + 1) * K])

            # ---------- code output ----------
            idx_i32 = res.tile([128, NT], I32, tag="idx_i32")
            nc.scalar.copy(out=idx_i32[:], in_=idx_all[:])
            code_ps = ps_sc.tile([128, 512], F32, space="PSUM", tag="sc")
            nc.tensor.transpose(out=code_ps[:NT, :128], in_=idx_all[:],
                                identity=ident[:])
            code_sb = res.tile([NT, 128], I32, tag="code_sb")
            nc.scalar.copy(out=code_sb[:], in_=code_ps[:NT, :128])
            nc.sync.dma_start(code_out[:], code_sb[:])

            # ---------- gather + loss tail ----------
            ls = res.tile([128, NT], F32, tag="ls")
            for t in range(NT):
                g = tailp.tile([128, D], F32, tag="g")
                nc.gpsimd.indirect_dma_start(
                    out=g[:], out_offset=None,
                    in_=table[:],
                    in_offset=bass.IndirectOffsetOnAxis(
                        ap=idx_i32[:, t:t + 1], axis=0),
                )
                nc.sync.dma_start(zq_out[t * 128:(t + 1) * 128, :], g[:])
                d16 = tailp.tile([128, D], F32, tag="d16")
                nc.vector.tensor_tensor(out=d16[:], in0=g[:],
                                        in1=z16r[:, t * D:(t + 1) * D],
                                        op=OP.subtract)
                dsq = tailp.tile([128, D], F32, tag="dsq")
                nc.scalar.activation(out=dsq[:], in_=d16[:], func=AF.Square,
                                     accum_out=ls[:, t:t + 1])

            TPB_TILES = NT // BATCHES_PER_CORE  # 32 tiles per batch
            inv = 1.0 / float(T * D)
            for b in range(BATCHES_PER_CORE):
                lb = res.tile([128, 1], F32, tag=f"lb{b}")
                nc.vector.tensor_reduce(
                    out=lb[:], in_=ls[:, b * TPB_TILES:(b + 1) * TPB_TILES],
                    axis=AX.X, op=OP.add)
                nc.gpsimd.partition_all_reduce(lb[:], lb[:], 128,
                                               bass_isa.ReduceOp.add)
                nc.vector.tensor_scalar(out=lb[:], in0=lb[:], scalar1=inv,
                                        scalar2=None, op0=OP.mult)
                nc.sync.dma_start(closs_out[:1, b:b + 1], lb[:1, :])

    nc.compile()
    return nc


def _prep_inputs(z, embedding, cluster_size, embedding_mean):
    z = np.ascontiguousarray(np.asarray(z, dtype=np.float32))
    E = np.ascontiguousarray(np.asarray(embedding, dtype=np.float32))
    cs = np.asarray(cluster_size, dtype=np.float32)
    em = np.asarray(embedding_mean, dtype=np.float32)

    E_hi = E.astype(np.float16)
    E_lo = (E - E_hi.astype(np.float32)).astype(np.float16)
    e2 = (E * E).sum(axis=0, dtype=np.float32).astype(np.float32)
    e2_hi = e2.astype(np.float16)
    e2_lo = (e2 - e2_hi.astype(np.float32)).astype(np.float16)
    emt = np.ascontiguousarray(em.T)

    z_flat = z.reshape(BT, D)
    in_maps = []
    for c in range(NCORES):
        zs = z_flat[c * TPC:(c + 1) * TPC]
        zt = np.ascontiguousarray((-2.0 * zs).T)          # [D, TPC] f32
        zt_hi = zt.astype(np.float16)
        zt_lo = (zt - zt_hi.astype(np.float32)).astype(np.float16)
        in_maps.append({
            "zt_hi": zt_hi,
            "zt_lo": zt_lo,
            "z16": zs.astype(np.float16),
            "e_hi": E_hi,
            "e_lo": E_lo,
            "e2_hi": e2_hi.reshape(1, K),
            "e2_lo": e2_lo.reshape(1, K),
            "emt": emt,
            "cs": cs.reshape(1, K),
        })
    return in_maps


def kernel(z, embedding, cluster_size, embedding_mean, _trace=False):
    if "nc" not in _CACHE:
        _CACHE["nc"] = _build()
    nc = _CACHE["nc"]
    in_maps = _prep_inputs(z, embedding, cluster_size, embedding_mean)
    r = run_bass_kernel_spmd(nc, in_maps, core_ids=list(range(NCORES)),
                             trace=_trace)
    _CACHE["last_run"] = r
    res = r.results

    z_q = np.concatenate([res[c]["zq"] for c in range(NCORES)], axis=0)
    z_q = z_q.reshape(B, T, D)
    code = np.concatenate([res[c]["code"].reshape(TPC) for c in range(NCORES)])
    code = code.reshape(B, T).astype(np.int32)
    closs = np.concatenate([res[c]["closs"].reshape(BATCHES_PER_CORE)
                            for c in range(NCORES)]).astype(np.float32)
    codebook_loss = np.zeros((B,), dtype=np.float32)
    newE = res[0]["newE"]
    ncs = res[0]["ncs"].reshape(K)
    nem = res[0]["nem"]
    return (z_q, closs, codebook_loss, code, newE, ncs, nem)
